# revision 10
# baseline (speedup 1.0000x reference)
"""Bilinear interpolation (affine grid sample) Trainium2 kernel.

Problem: X [16, 256, 256, 32] f32, t [16, 6] affine params ->
out[b, i, j, :] = bilinear sample of X[b] at affine-transformed grid points
(matching the oracle's semantics on this jax backend, including its
round-to-nearest-even f32->i32 cast).

Sharding: pure data parallel over batch; 2 batches per core on 8 cores.

Per core:
  - host replicates the oracle's tiny coordinate pipeline with the same jax
    ops (bitwise-identical x/y/x0/y0) and derives wrapped int16 gather block
    indices; X is padded by one 256B block.
  - device gathers 512B 4-pixel blocks via dma_gather (row y0 and row y1
    per output pixel), one 512B descriptor per block.
  - lerp weights are computed on DVE and routed onto the 3 possible pixel
    slots of each gathered block via indicator masks (this reproduces the
    reference's clip behavior exactly); weighted blocks are summed on the
    PE via identity-matmul PSUM accumulation and stored.

Output pixel mapping: gather ordinal n -> SBUF (partition n%128, slot
n//128); we choose pixel(n) = (n%128)*512 + n//128 so each partition holds
a contiguous 512-pixel range per batch and writeback DMAs are contiguous.
"""
import sys

sys.path.insert(0, "/opt/trn_rl_repo")

import numpy as np

import concourse.bass as bass
import concourse.bacc as bacc
import concourse.mybir as mybir
import concourse.tile as tile
from concourse.masks import make_identity

f32 = mybir.dt.float32
i32 = mybir.dt.int32
i16 = mybir.dt.int16
OP = mybir.AluOpType
ACT = mybir.ActivationFunctionType

P = 128          # SBUF partitions
B_LOCAL = 2      # batches per core
H = W = 256      # input image dims
C = 32           # channels
HO = WO = 256    # output dims
HWO = HO * WO    # 65536 pixels per batch
NJ = B_LOCAL * HWO // P   # 1024 slots (both batches)
SJ = HWO // P    # 512 slots per batch
CH = 16          # slots per gather chunk (PSUM free dim = CH*C = 512)
NCH = NJ // CH   # 64 chunks
NIDX = CH * P    # 2048 gathered blocks per chunk per pair
BATCH_ELEMS = HWO * C      # 2097152
PAD = 64         # f32 elems of padding after X (one gather block overrun)
N_CORES = 8


def _bcast(ap, n):
    """Append a step-0 dim of size n to an AP (inner broadcast)."""
    return bass.AP(ap.tensor, ap.offset, list(ap.ap) + [[0, n]])


def build_nc():
    nc = bacc.Bacc("TRN2", target_bir_lowering=False, debug=False)

    X = nc.dram_tensor("X", [B_LOCAL * BATCH_ELEMS + PAD], f32,
                       kind="ExternalInput")
    XF = nc.dram_tensor("xf", [B_LOCAL * HWO], f32, kind="ExternalInput")
    YF = nc.dram_tensor("yf", [B_LOCAL * HWO], f32, kind="ExternalInput")
    X0I = nc.dram_tensor("x0i", [B_LOCAL * HWO], i32, kind="ExternalInput")
    Y0I = nc.dram_tensor("y0i", [B_LOCAL * HWO], i32, kind="ExternalInput")
    # wrapped int16 block indices (see host_aux): [pair, batch, 128, SJ*8]
    IW = nc.dram_tensor("iw", [2, B_LOCAL, P, SJ * 8], i16, kind="ExternalInput")
    OUT = nc.dram_tensor("out", [B_LOCAL * HWO, C], f32, kind="ExternalOutput")

    def coord_view(dram):
        # dram flat [b*HWO + p*SJ + s] -> AP [p, b, s] matching tile [128, NJ]
        a = dram.ap()
        return bass.AP(a.tensor, 0, [[SJ, P], [HWO, B_LOCAL], [1, SJ]])

    # out viewed [b, p, s, c]: global row = b*65536 + p*512 + s
    out_v = OUT.ap().rearrange("(b p s) c -> b p s c", b=B_LOCAL, p=P)

    with tile.TileContext(nc) as tc:
        with (
            tc.tile_pool(name="persist", bufs=1) as pp,
            tc.tile_pool(name="scratch", bufs=1) as sp,
            tc.tile_pool(name="chunk", bufs=2) as cp,
            tc.tile_pool(name="mtiles", bufs=2) as mp,
            tc.tile_pool(name="psum", bufs=4, space="PSUM") as qp,
        ):
            # scratch registers (reused [P, NJ] f32 tiles)
            def reg(name, dt=f32):
                return sp.tile([P, NJ], dt, name=name, tag=name)

            rA, rB, rC, rD, rE, rF = (reg(n) for n in "rA rB rC rD rE rF".split())
            rG, rHh, rI, rK = (reg(n) for n in "rG rHh rI rK".split())
            rII = reg("rII", i32)

            V = nc.vector
            S = nc.scalar

            ident = pp.tile([P, P], f32, name="ident", tag="ident")
            make_identity(nc, ident[:])

            # wrapped gather indices, resident in SBUF
            iw = {}
            for pi, pair in enumerate(("AC", "BD")):
                t_ = pp.tile([P, B_LOCAL * SJ * 8], i16, name="iw" + pair,
                             tag="iw" + pair)
                for b in range(B_LOCAL):
                    nc.sync.dma_start(t_[:, b * SJ * 8:(b + 1) * SJ * 8],
                                      IW.ap()[pi, b])
                iw[pair] = t_

            # ---- load host-computed coords: x, y (f32), x0, y0 (round int) ----
            nc.sync.dma_start(rA[:].rearrange("p (b s) -> p b s", b=B_LOCAL),
                              coord_view(XF))                  # A = x
            nc.sync.dma_start(rD[:].rearrange("p (b s) -> p b s", b=B_LOCAL),
                              coord_view(YF))                  # D = y
            nc.sync.dma_start(rII[:].rearrange("p (b s) -> p b s", b=B_LOCAL),
                              coord_view(X0I))
            V.tensor_copy(rB[:], rII[:])                       # B = x0 (float)
            nc.sync.dma_start(rII[:].rearrange("p (b s) -> p b s", b=B_LOCAL),
                              coord_view(Y0I))
            V.tensor_copy(rC[:], rII[:])                       # C = y0 (float)

            # ---- clips ----
            V.tensor_scalar(rE[:], rB[:], 0.0, float(W - 1), OP.max, OP.min)  # E=x0c
            V.tensor_scalar(rF[:], rB[:], 1.0, float(W - 1), OP.add, OP.min)
            V.tensor_scalar(rF[:], rF[:], 0.0, None, OP.max)                  # F=x1c
            V.tensor_scalar(rG[:], rE[:], float(W - 2), None, OP.min)         # G=xg
            V.tensor_scalar(rB[:], rC[:], 0.0, float(H - 1), OP.max, OP.min)  # B=y0c
            V.tensor_scalar(rHh[:], rC[:], 1.0, float(H - 1), OP.add, OP.min)
            V.tensor_scalar(rHh[:], rHh[:], 0.0, None, OP.max)                # Hh=y1c

            # ---- lerp deltas & weights ----
            V.tensor_tensor(rC[:], rF[:], rA[:], OP.subtract)   # C = hx0 = x1c-x
            V.tensor_tensor(rA[:], rA[:], rE[:], OP.subtract)   # A = hx1 = x-x0c
            V.tensor_tensor(rI[:], rHh[:], rD[:], OP.subtract)  # I = vy0 = y1c-y
            V.tensor_tensor(rD[:], rD[:], rB[:], OP.subtract)   # D = vy1 = y-y0c
            # rB = y0c, rHh = y1c no longer needed (indices precomputed on host)

            V.tensor_tensor(rB[:], rC[:], rI[:], OP.mult)       # B = wa
            V.tensor_tensor(rHh[:], rC[:], rD[:], OP.mult)      # Hh = wb
            V.tensor_tensor(rC[:], rA[:], rI[:], OP.mult)       # C = wc
            V.tensor_tensor(rA[:], rA[:], rD[:], OP.mult)       # A = wd

            # ---- slot positions within the gathered 4-pixel block ----
            # block pixel start = xg - (xg mod 2); sA = x0c-xg+m2, sC = x1c-xg+m2
            # m2 = xg - 2*floor(xg/2); floor(xg/2) = rne(0.5*xg - 0.25) via 2^23
            MAGIC = 8388608.0
            V.tensor_scalar(rI[:], rG[:], 0.5, -0.25, OP.mult, OP.add)
            V.tensor_scalar(rI[:], rI[:], MAGIC, None, OP.add)
            V.tensor_scalar(rI[:], rI[:], MAGIC, None, OP.subtract)  # floor(xg/2)
            V.scalar_tensor_tensor(rI[:], rI[:], -2.0, rG[:],
                                   OP.mult, OP.add)             # I = m2
            V.tensor_tensor(rD[:], rE[:], rG[:], OP.subtract)
            V.tensor_tensor(rD[:], rD[:], rI[:], OP.add)        # D = sA in {0,1,2}
            V.tensor_tensor(rE[:], rF[:], rG[:], OP.subtract)
            V.tensor_tensor(rE[:], rE[:], rI[:], OP.add)        # E = sC in {0,1,2}

            # ---- slot-routed weights: w<pair><k> = w_lo*ind(sA==k)+w_hi*ind(sC==k)
            wsel = {}
            for k in range(3):
                V.tensor_scalar(rF[:], rD[:], float(k), None, OP.is_equal)  # iA_k
                V.tensor_scalar(rG[:], rE[:], float(k), None, OP.is_equal)  # iC_k
                for pair, (w_lo, w_hi) in (("AC", (rB, rC)), ("BD", (rHh, rA))):
                    wt = pp.tile([P, NJ], f32, name=f"w{pair}{k}", tag=f"w{pair}{k}")
                    V.tensor_tensor(wt[:], w_lo[:], rF[:], OP.mult)
                    V.tensor_tensor(rK[:], w_hi[:], rG[:], OP.mult)
                    V.tensor_tensor(wt[:], wt[:], rK[:], OP.add)
                    wsel[pair, k] = wt

            # ---- gather + weighted-sum chunks ----
            for ch in range(NCH):
                b = ch // (NCH // B_LOCAL)
                co = ch * CH                 # global slot base
                sl = slice(co, co + CH)
                f0 = (co - b * SJ) * 8       # wrapped free offset within batch
                g = {}
                for pair in ("AC", "BD"):
                    gt = cp.tile([P, CH, 4 * C], f32, name="g" + pair,
                                 tag="g" + pair)
                    in_ap = bass.AP(X.ap().tensor, b * BATCH_ELEMS,
                                    [[2 * C, BATCH_ELEMS // (2 * C)], [1, 4 * C]])
                    nc.gpsimd.dma_gather(
                        out_ap=gt[:],
                        in_ap=in_ap,
                        idxs_ap=iw[pair][:, b * SJ * 8 + f0:
                                         b * SJ * 8 + f0 + CH * 8],
                        num_idxs=NIDX,
                        num_idxs_reg=NIDX,
                        elem_size=4 * C,
                        elem_step=2 * C,
                        single_packet=False,
                    )
                    g[pair] = gt

                ps = qp.tile([P, CH * C], f32, name="ps", tag="ps")
                first = True
                for pair in ("AC", "BD"):
                    for k in range(3):
                        m = mp.tile([P, CH, C], f32, name=f"m{pair}{k}",
                                    tag=f"m{pair}{k}")
                        V.tensor_tensor(m[:], g[pair][:, :, C * k:C * (k + 1)],
                                        _bcast(wsel[pair, k][:, sl], C), OP.mult)
                        nc.tensor.matmul(out=ps[:], lhsT=ident[:],
                                         rhs=m[:].rearrange("p a b -> p (a b)"),
                                         start=first, stop=(pair == "BD" and k == 2))
                        first = False

                osb = cp.tile([P, CH * C], f32, name="osb", tag="osb")
                S.activation(osb[:], ps[:], ACT.Copy)
                nc.sync.dma_start(out_v[b, :, co - b * SJ:co - b * SJ + CH, :],
                                  osb[:])

    nc.compile()
    return nc


_NC_CACHE = []


def _get_nc():
    if not _NC_CACHE:
        _NC_CACHE.append(build_nc())
    return _NC_CACHE[0]


def host_coords(t):
    """Replicate the oracle's coordinate pipeline with the same jax ops on
    the same backend, so x/y/x0/y0 are bitwise identical to the oracle's."""
    import jax.numpy as jnp

    B = t.shape[0]
    xs = jnp.linspace(-1.0, 1.0, WO, dtype=jnp.float32)
    ys = jnp.linspace(-1.0, 1.0, HO, dtype=jnp.float32)
    xc, yc = jnp.meshgrid(xs, ys)
    grid = jnp.stack([xc.ravel(), yc.ravel(), jnp.ones(HWO, jnp.float32)], axis=0)
    theta = jnp.asarray(t).reshape(B, 2, 3)
    sampled = jnp.einsum('bij,jk->bik', theta, grid)
    x = sampled[:, 0, :].reshape(-1)
    y = sampled[:, 1, :].reshape(-1)
    x = 0.5 * (x + 1.0) * jnp.float32(H)
    y = 0.5 * (y + 1.0) * jnp.float32(W)
    x0 = x.astype(jnp.int32)
    y0 = y.astype(jnp.int32)
    return (np.asarray(x), np.asarray(y), np.asarray(x0), np.asarray(y0))


def host_aux(t):
    """Host-side: coords (bitwise oracle replica) + wrapped int16 gather
    indices. Returns dict of per-core input arrays (leading dim N_CORES)."""
    B = t.shape[0]
    xf, yf, x0i, y0i = host_coords(t)

    x0c = np.clip(x0i, 0, W - 1)
    xg = np.minimum(x0c, W - 2)
    y0c = np.clip(y0i, 0, H - 1)
    y1c = np.clip(y0i + 1, 0, H - 1)
    kAC = (y0c * (W // 2) + (xg >> 1)).astype(np.int64)   # block idx, 0..32767
    kBD = (y1c * (W // 2) + (xg >> 1)).astype(np.int64)

    def wrap(k):
        # k: [B*HWO] in pixel-linear order; per batch build wrapped int16:
        # ordinal n = j*128 + p maps to pixel p*SJ + j;
        # wrapped[q, f] = k_by_n[f*16 + q%16], replicated over 8 cores.
        k2 = k.reshape(B, P, SJ)
        k_by_n = np.transpose(k2, (0, 2, 1)).reshape(B, HWO)  # [B, n]
        w = k_by_n.reshape(B, HWO // 16, 16).transpose(0, 2, 1)  # [B, 16, n//16]
        w = np.tile(w, (1, 8, 1)).astype(np.int16)               # [B, 128, n//16]
        return w.reshape(N_CORES, B_LOCAL, P, SJ * 8)

    return {
        "xf": np.ascontiguousarray(xf.reshape(N_CORES, B_LOCAL * HWO)),
        "yf": np.ascontiguousarray(yf.reshape(N_CORES, B_LOCAL * HWO)),
        "x0i": np.ascontiguousarray(x0i.reshape(N_CORES, B_LOCAL * HWO)),
        "y0i": np.ascontiguousarray(y0i.reshape(N_CORES, B_LOCAL * HWO)),
        "iw": np.ascontiguousarray(
            np.stack([wrap(kAC), wrap(kBD)], axis=1)),  # [cores, 2, BL, P, SJ*8]
    }


def shard_X(X):
    Xs = X.reshape(N_CORES, B_LOCAL * BATCH_ELEMS)
    return np.concatenate(
        [Xs, np.zeros((N_CORES, PAD), np.float32)], axis=1)


def kernel(X, t):
    from concourse import bass_utils

    nc = _get_nc()
    B = X.shape[0]
    assert B == N_CORES * B_LOCAL
    aux = host_aux(t)
    Xp = shard_X(np.ascontiguousarray(X))
    in_maps = [dict(X=Xp[i], **{k: v[i] for k, v in aux.items()})
               for i in range(N_CORES)]
    res = bass_utils.run_bass_kernel_spmd(nc, in_maps, core_ids=list(range(N_CORES)))
    out = np.stack([res.results[i]["out"] for i in range(N_CORES)])
    return out.reshape(B, HO, WO, C)


# revision 11
# speedup vs baseline: 16.9406x; 16.9406x over previous
"""Bilinear interpolation (affine grid sample) Trainium2 kernel.

Problem: X [16, 256, 256, 32] f32, t [16, 6] affine params ->
out[b, i, j, :] = bilinear sample of X[b] at affine-transformed grid points
(matching the oracle's semantics on this jax backend, including its
round-to-nearest-even f32->i32 cast).

Sharding: pure data parallel over batch; 2 batches per core on 8 cores.

Per core:
  - host replicates the oracle's tiny coordinate pipeline with the same jax
    ops (bitwise-identical x/y/x0/y0) and derives wrapped int16 gather block
    indices; X is padded by one 256B block.
  - device gathers 512B 4-pixel blocks via dma_gather (row y0 and row y1
    per output pixel), one 512B descriptor per block.
  - lerp weights are computed on DVE and routed onto the 3 possible pixel
    slots of each gathered block via indicator masks (this reproduces the
    reference's clip behavior exactly); weighted blocks are summed on the
    PE via identity-matmul PSUM accumulation and stored.

Output pixel mapping: gather ordinal n -> SBUF (partition n%128, slot
n//128); we choose pixel(n) = (n%128)*512 + n//128 so each partition holds
a contiguous 512-pixel range per batch and writeback DMAs are contiguous.
"""
import sys

sys.path.insert(0, "/opt/trn_rl_repo")

import numpy as np

import concourse.bass as bass
import concourse.bacc as bacc
import concourse.mybir as mybir
import concourse.tile as tile
from concourse.masks import make_identity

f32 = mybir.dt.float32
i32 = mybir.dt.int32
i16 = mybir.dt.int16
OP = mybir.AluOpType
ACT = mybir.ActivationFunctionType

P = 128          # SBUF partitions
B_LOCAL = 2      # batches per core
H = W = 256      # input image dims
C = 32           # channels
HO = WO = 256    # output dims
HWO = HO * WO    # 65536 pixels per batch
NJ = B_LOCAL * HWO // P   # 1024 slots (both batches)
SJ = HWO // P    # 512 slots per batch
CH = 16          # slots per gather chunk (PSUM free dim = CH*C = 512)
NCH = NJ // CH   # 64 chunks
NIDX = CH * P    # 2048 gathered blocks per chunk per pair
BATCH_ELEMS = HWO * C      # 2097152
PAD = 64         # f32 elems of padding after X (one gather block overrun)
N_CORES = 8


def _bcast(ap, n):
    """Append a step-0 dim of size n to an AP (inner broadcast)."""
    return bass.AP(ap.tensor, ap.offset, list(ap.ap) + [[0, n]])


def build_nc(repeat=1):
    nc = bacc.Bacc("TRN2", target_bir_lowering=False, debug=False)

    X = nc.dram_tensor("X", [B_LOCAL * BATCH_ELEMS + PAD], f32,
                       kind="ExternalInput")
    XF = nc.dram_tensor("xf", [B_LOCAL * HWO], f32, kind="ExternalInput")
    YF = nc.dram_tensor("yf", [B_LOCAL * HWO], f32, kind="ExternalInput")
    X0I = nc.dram_tensor("x0i", [B_LOCAL * HWO], i32, kind="ExternalInput")
    Y0I = nc.dram_tensor("y0i", [B_LOCAL * HWO], i32, kind="ExternalInput")
    # wrapped int16 block indices (see host_aux): [pair, batch, 128, SJ*8]
    IW = nc.dram_tensor("iw", [2, B_LOCAL, P, SJ * 8], i16, kind="ExternalInput")
    OUT = nc.dram_tensor("out", [B_LOCAL * HWO, C], f32, kind="ExternalOutput")

    def coord_view(dram):
        # dram flat [b*HWO + p*SJ + s] -> AP [p, b, s] matching tile [128, NJ]
        a = dram.ap()
        return bass.AP(a.tensor, 0, [[SJ, P], [HWO, B_LOCAL], [1, SJ]])

    # out viewed [b, p, s, c]: global row = b*65536 + p*512 + s
    out_v = OUT.ap().rearrange("(b p s) c -> b p s c", b=B_LOCAL, p=P)

    with tile.TileContext(nc) as tc:
        with (
            tc.tile_pool(name="persist", bufs=1) as pp,
            tc.tile_pool(name="scratch", bufs=1) as sp,
            tc.tile_pool(name="chunk", bufs=2) as cp,
            tc.tile_pool(name="mtiles", bufs=2) as mp,
            tc.tile_pool(name="psum", bufs=4, space="PSUM") as qp,
        ):
            # scratch registers (reused [P, NJ] f32 tiles)
            def reg(name, dt=f32):
                return sp.tile([P, NJ], dt, name=name, tag=name)

            rA, rB, rC, rD, rE, rF = (reg(n) for n in "rA rB rC rD rE rF".split())
            rG, rHh, rI, rK = (reg(n) for n in "rG rHh rI rK".split())
            rII = reg("rII", i32)

            V = nc.vector
            S = nc.scalar

            ident = pp.tile([P, P], f32, name="ident", tag="ident")
            make_identity(nc, ident[:])
            wsel_store = {}

            # wrapped gather indices, resident in SBUF
            iw = {}
            for pi, pair in enumerate(("AC", "BD")):
                t_ = pp.tile([P, B_LOCAL * SJ * 8], i16, name="iw" + pair,
                             tag="iw" + pair)
                for b in range(B_LOCAL):
                    nc.sync.dma_start(t_[:, b * SJ * 8:(b + 1) * SJ * 8],
                                      IW.ap()[pi, b])
                iw[pair] = t_

            # ---- load host-computed coords: x, y (f32), x0, y0 (round int) ----
            nc.sync.dma_start(rA[:].rearrange("p (b s) -> p b s", b=B_LOCAL),
                              coord_view(XF))                  # A = x
            nc.sync.dma_start(rD[:].rearrange("p (b s) -> p b s", b=B_LOCAL),
                              coord_view(YF))                  # D = y
            nc.sync.dma_start(rII[:].rearrange("p (b s) -> p b s", b=B_LOCAL),
                              coord_view(X0I))
            V.tensor_copy(rB[:], rII[:])                       # B = x0 (float)
            nc.sync.dma_start(rII[:].rearrange("p (b s) -> p b s", b=B_LOCAL),
                              coord_view(Y0I))
            V.tensor_copy(rC[:], rII[:])                       # C = y0 (float)

            # ---- clips ----
            V.tensor_scalar(rE[:], rB[:], 0.0, float(W - 1), OP.max, OP.min)  # E=x0c
            V.tensor_scalar(rF[:], rB[:], 1.0, float(W - 1), OP.add, OP.min)
            V.tensor_scalar(rF[:], rF[:], 0.0, None, OP.max)                  # F=x1c
            V.tensor_scalar(rG[:], rE[:], float(W - 2), None, OP.min)         # G=xg
            V.tensor_scalar(rB[:], rC[:], 0.0, float(H - 1), OP.max, OP.min)  # B=y0c
            V.tensor_scalar(rHh[:], rC[:], 1.0, float(H - 1), OP.add, OP.min)
            V.tensor_scalar(rHh[:], rHh[:], 0.0, None, OP.max)                # Hh=y1c

            # ---- lerp deltas & weights ----
            V.tensor_tensor(rC[:], rF[:], rA[:], OP.subtract)   # C = hx0 = x1c-x
            V.tensor_tensor(rA[:], rA[:], rE[:], OP.subtract)   # A = hx1 = x-x0c
            V.tensor_tensor(rI[:], rHh[:], rD[:], OP.subtract)  # I = vy0 = y1c-y
            V.tensor_tensor(rD[:], rD[:], rB[:], OP.subtract)   # D = vy1 = y-y0c
            # rB = y0c, rHh = y1c no longer needed (indices precomputed on host)

            V.tensor_tensor(rB[:], rC[:], rI[:], OP.mult)       # B = wa
            V.tensor_tensor(rHh[:], rC[:], rD[:], OP.mult)      # Hh = wb
            V.tensor_tensor(rC[:], rA[:], rI[:], OP.mult)       # C = wc
            V.tensor_tensor(rA[:], rA[:], rD[:], OP.mult)       # A = wd

            # ---- slot positions within the gathered 4-pixel block ----
            # block pixel start = xg - (xg mod 2); sA = x0c-xg+m2, sC = x1c-xg+m2
            # m2 = xg - 2*floor(xg/2); floor(xg/2) = rne(0.5*xg - 0.25) via 2^23
            MAGIC = 8388608.0
            V.tensor_scalar(rI[:], rG[:], 0.5, -0.25, OP.mult, OP.add)
            V.tensor_scalar(rI[:], rI[:], MAGIC, None, OP.add)
            V.tensor_scalar(rI[:], rI[:], MAGIC, None, OP.subtract)  # floor(xg/2)
            V.scalar_tensor_tensor(rI[:], rI[:], -2.0, rG[:],
                                   OP.mult, OP.add)             # I = m2
            V.tensor_tensor(rD[:], rE[:], rG[:], OP.subtract)
            V.tensor_tensor(rD[:], rD[:], rI[:], OP.add)        # D = sA in {0,1,2}
            V.tensor_tensor(rE[:], rF[:], rG[:], OP.subtract)
            V.tensor_tensor(rE[:], rE[:], rI[:], OP.add)        # E = sC in {0,1,2}

            # ---- slot-routed weights: w<pair><k> = w_lo*ind(sA==k)+w_hi*ind(sC==k)
            wsel = wsel_store
            wsel["ident"] = ident
            for k in range(3):
                V.tensor_scalar(rF[:], rD[:], float(k), None, OP.is_equal)  # iA_k
                V.tensor_scalar(rG[:], rE[:], float(k), None, OP.is_equal)  # iC_k
                for pair, (w_lo, w_hi) in (("AC", (rB, rC)), ("BD", (rHh, rA))):
                    wt = pp.tile([P, NJ], f32, name=f"w{pair}{k}", tag=f"w{pair}{k}")
                    V.tensor_tensor(wt[:], w_lo[:], rF[:], OP.mult)
                    V.tensor_tensor(rK[:], w_hi[:], rG[:], OP.mult)
                    V.tensor_tensor(wt[:], wt[:], rK[:], OP.add)
                    wsel[pair, k] = wt

            # ---- gather + weighted-sum chunks ----
            import contextlib
            loop_cm = tc.For_i(0, repeat, 1) if repeat > 1 else \
                contextlib.nullcontext()
            with loop_cm:
                chunk_body(nc, tc, cp, mp, qp, iw, wsel, X, out_v)

    nc.compile()
    return nc


def chunk_body(nc, tc, cp, mp, qp, iw, wsel, X, out_v):
            V = nc.vector
            S = nc.scalar
            ident = wsel["ident"]
            for ch in range(NCH):
                b = ch // (NCH // B_LOCAL)
                co = ch * CH                 # global slot base
                sl = slice(co, co + CH)
                f0 = (co - b * SJ) * 8       # wrapped free offset within batch
                g = {}
                for pair in ("AC", "BD"):
                    gt = cp.tile([P, CH, 4 * C], f32, name="g" + pair,
                                 tag="g" + pair)
                    in_ap = bass.AP(X.ap().tensor, b * BATCH_ELEMS,
                                    [[2 * C, BATCH_ELEMS // (2 * C)], [1, 4 * C]])
                    nc.gpsimd.dma_gather(
                        out_ap=gt[:],
                        in_ap=in_ap,
                        idxs_ap=iw[pair][:, b * SJ * 8 + f0:
                                         b * SJ * 8 + f0 + CH * 8],
                        num_idxs=NIDX,
                        num_idxs_reg=NIDX,
                        elem_size=4 * C,
                        elem_step=2 * C,
                        single_packet=False,
                    )
                    g[pair] = gt

                ps = qp.tile([P, CH * C], f32, name="ps", tag="ps")
                first = True
                for pair in ("AC", "BD"):
                    for k in range(3):
                        m = mp.tile([P, CH, C], f32, name=f"m{pair}{k}",
                                    tag=f"m{pair}{k}")
                        V.tensor_tensor(m[:], g[pair][:, :, C * k:C * (k + 1)],
                                        _bcast(wsel[pair, k][:, sl], C), OP.mult)
                        nc.tensor.matmul(out=ps[:], lhsT=ident[:],
                                         rhs=m[:].rearrange("p a b -> p (a b)"),
                                         start=first, stop=(pair == "BD" and k == 2))
                        first = False

                osb = cp.tile([P, CH * C], f32, name="osb", tag="osb")
                S.activation(osb[:], ps[:], ACT.Copy)
                nc.sync.dma_start(out_v[b, :, co - b * SJ:co - b * SJ + CH, :],
                                  osb[:])


_NC_CACHE = []


def _get_nc():
    if not _NC_CACHE:
        _NC_CACHE.append(build_nc())
    return _NC_CACHE[0]


def host_coords(t):
    """Replicate the oracle's coordinate pipeline with the same jax ops on
    the same backend, so x/y/x0/y0 are bitwise identical to the oracle's."""
    import jax.numpy as jnp

    B = t.shape[0]
    xs = jnp.linspace(-1.0, 1.0, WO, dtype=jnp.float32)
    ys = jnp.linspace(-1.0, 1.0, HO, dtype=jnp.float32)
    xc, yc = jnp.meshgrid(xs, ys)
    grid = jnp.stack([xc.ravel(), yc.ravel(), jnp.ones(HWO, jnp.float32)], axis=0)
    theta = jnp.asarray(t).reshape(B, 2, 3)
    sampled = jnp.einsum('bij,jk->bik', theta, grid)
    x = sampled[:, 0, :].reshape(-1)
    y = sampled[:, 1, :].reshape(-1)
    x = 0.5 * (x + 1.0) * jnp.float32(H)
    y = 0.5 * (y + 1.0) * jnp.float32(W)
    x0 = x.astype(jnp.int32)
    y0 = y.astype(jnp.int32)
    return (np.asarray(x), np.asarray(y), np.asarray(x0), np.asarray(y0))


def host_aux(t):
    """Host-side: coords (bitwise oracle replica) + wrapped int16 gather
    indices. Returns dict of per-core input arrays (leading dim N_CORES)."""
    B = t.shape[0]
    xf, yf, x0i, y0i = host_coords(t)

    x0c = np.clip(x0i, 0, W - 1)
    xg = np.minimum(x0c, W - 2)
    y0c = np.clip(y0i, 0, H - 1)
    y1c = np.clip(y0i + 1, 0, H - 1)
    kAC = (y0c * (W // 2) + (xg >> 1)).astype(np.int64)   # block idx, 0..32767
    kBD = (y1c * (W // 2) + (xg >> 1)).astype(np.int64)

    def wrap(k):
        # k: [B*HWO] in pixel-linear order; per batch build wrapped int16:
        # ordinal n = j*128 + p maps to pixel p*SJ + j;
        # wrapped[q, f] = k_by_n[f*16 + q%16], replicated over 8 cores.
        k2 = k.reshape(B, P, SJ)
        k_by_n = np.transpose(k2, (0, 2, 1)).reshape(B, HWO)  # [B, n]
        w = k_by_n.reshape(B, HWO // 16, 16).transpose(0, 2, 1)  # [B, 16, n//16]
        w = np.tile(w, (1, 8, 1)).astype(np.int16)               # [B, 128, n//16]
        return w.reshape(N_CORES, B_LOCAL, P, SJ * 8)

    return {
        "xf": np.ascontiguousarray(xf.reshape(N_CORES, B_LOCAL * HWO)),
        "yf": np.ascontiguousarray(yf.reshape(N_CORES, B_LOCAL * HWO)),
        "x0i": np.ascontiguousarray(x0i.reshape(N_CORES, B_LOCAL * HWO)),
        "y0i": np.ascontiguousarray(y0i.reshape(N_CORES, B_LOCAL * HWO)),
        "iw": np.ascontiguousarray(
            np.stack([wrap(kAC), wrap(kBD)], axis=1)),  # [cores, 2, BL, P, SJ*8]
    }


def shard_X(X):
    Xs = X.reshape(N_CORES, B_LOCAL * BATCH_ELEMS)
    return np.concatenate(
        [Xs, np.zeros((N_CORES, PAD), np.float32)], axis=1)


def kernel(X, t):
    from concourse import bass_utils

    nc = _get_nc()
    B = X.shape[0]
    assert B == N_CORES * B_LOCAL
    aux = host_aux(t)
    Xp = shard_X(np.ascontiguousarray(X))
    in_maps = [dict(X=Xp[i], **{k: v[i] for k, v in aux.items()})
               for i in range(N_CORES)]
    res = bass_utils.run_bass_kernel_spmd(nc, in_maps, core_ids=list(range(N_CORES)))
    out = np.stack([res.results[i]["out"] for i in range(N_CORES)])
    return out.reshape(B, HO, WO, C)


# revision 13
# speedup vs baseline: 44.9239x; 2.6518x over previous
"""Bilinear interpolation (affine grid sample) Trainium2 kernel.

Problem: X [16, 256, 256, 32] f32, t [16, 6] affine params ->
out[b, i, j, :] = bilinear sample of X[b] at affine-transformed grid points
(matching the oracle's semantics on this jax backend, including its
round-to-nearest-even f32->i32 cast).

Sharding: pure data parallel over batch; 2 batches per core on 8 cores.

Per core:
  - host replicates the oracle's tiny coordinate pipeline with the same jax
    ops (bitwise-identical x/y/x0/y0) and derives wrapped int16 gather block
    indices; X is padded by one 256B block.
  - device gathers 512B 4-pixel blocks via dma_gather (row y0 and row y1
    per output pixel), one 512B descriptor per block.
  - lerp weights are computed on DVE and routed onto the 3 possible pixel
    slots of each gathered block via indicator masks (this reproduces the
    reference's clip behavior exactly); weighted blocks are summed on the
    PE via identity-matmul PSUM accumulation and stored.

Output pixel mapping: gather ordinal n -> SBUF (partition n%128, slot
n//128); we choose pixel(n) = (n%128)*512 + n//128 so each partition holds
a contiguous 512-pixel range per batch and writeback DMAs are contiguous.
"""
import sys

sys.path.insert(0, "/opt/trn_rl_repo")

import numpy as np

import concourse.bass as bass
import concourse.bacc as bacc
import concourse.mybir as mybir
import concourse.tile as tile
from concourse.masks import make_identity

f32 = mybir.dt.float32
i32 = mybir.dt.int32
i16 = mybir.dt.int16
OP = mybir.AluOpType
ACT = mybir.ActivationFunctionType

P = 128          # SBUF partitions
B_LOCAL = 2      # batches per core
H = W = 256      # input image dims
C = 32           # channels
HO = WO = 256    # output dims
HWO = HO * WO    # 65536 pixels per batch
NJ = B_LOCAL * HWO // P   # 1024 slots (both batches)
SJ = HWO // P    # 512 slots per batch
CH = 16          # slots per gather chunk (PSUM free dim = CH*C = 512)
NCH = NJ // CH   # 64 chunks
NIDX = CH * P    # 2048 gathered blocks per chunk per pair
BATCH_ELEMS = HWO * C      # 2097152
PAD = 64         # f32 elems of padding after X (one gather block overrun)
N_CORES = 8


def _bcast(ap, n):
    """Append a step-0 dim of size n to an AP (inner broadcast)."""
    return bass.AP(ap.tensor, ap.offset, list(ap.ap) + [[0, n]])


def build_nc(repeat=1, nq=4):
    nc = bacc.Bacc("TRN2", target_bir_lowering=False, debug=False,
                   num_swdge_queues=nq)

    X = nc.dram_tensor("X", [B_LOCAL * BATCH_ELEMS + PAD], f32,
                       kind="ExternalInput")
    XF = nc.dram_tensor("xf", [B_LOCAL * HWO], f32, kind="ExternalInput")
    YF = nc.dram_tensor("yf", [B_LOCAL * HWO], f32, kind="ExternalInput")
    X0I = nc.dram_tensor("x0i", [B_LOCAL * HWO], i32, kind="ExternalInput")
    Y0I = nc.dram_tensor("y0i", [B_LOCAL * HWO], i32, kind="ExternalInput")
    # wrapped int16 block indices (see host_aux): [pair, batch, 128, SJ*8]
    IW = nc.dram_tensor("iw", [2, B_LOCAL, P, SJ * 8], i16, kind="ExternalInput")
    OUT = nc.dram_tensor("out", [B_LOCAL * HWO, C], f32, kind="ExternalOutput")

    def coord_view(dram):
        # dram flat [b*HWO + p*SJ + s] -> AP [p, b, s] matching tile [128, NJ]
        a = dram.ap()
        return bass.AP(a.tensor, 0, [[SJ, P], [HWO, B_LOCAL], [1, SJ]])

    # out viewed [b, p, s, c]: global row = b*65536 + p*512 + s
    out_v = OUT.ap().rearrange("(b p s) c -> b p s c", b=B_LOCAL, p=P)

    with tile.TileContext(nc) as tc:
        with (
            tc.tile_pool(name="persist", bufs=1) as pp,
            tc.tile_pool(name="scratch", bufs=1) as sp,
            tc.tile_pool(name="chunk", bufs=2) as cp,
            tc.tile_pool(name="mtiles", bufs=2) as mp,
            tc.tile_pool(name="psum", bufs=4, space="PSUM") as qp,
        ):
            # scratch registers (reused [P, NJ] f32 tiles)
            def reg(name, dt=f32):
                return sp.tile([P, NJ], dt, name=name, tag=name)

            rA, rB, rC, rD, rE, rF = (reg(n) for n in "rA rB rC rD rE rF".split())
            rG, rHh, rI, rK = (reg(n) for n in "rG rHh rI rK".split())
            rII = reg("rII", i32)

            V = nc.vector
            S = nc.scalar

            ident = pp.tile([P, P], f32, name="ident", tag="ident")
            make_identity(nc, ident[:])
            wsel_store = {}

            # wrapped gather indices, resident in SBUF
            iw = {}
            for pi, pair in enumerate(("AC", "BD")):
                t_ = pp.tile([P, B_LOCAL * SJ * 8], i16, name="iw" + pair,
                             tag="iw" + pair)
                for b in range(B_LOCAL):
                    nc.sync.dma_start(t_[:, b * SJ * 8:(b + 1) * SJ * 8],
                                      IW.ap()[pi, b])
                iw[pair] = t_

            # ---- load host-computed coords: x, y (f32), x0, y0 (round int) ----
            nc.sync.dma_start(rA[:].rearrange("p (b s) -> p b s", b=B_LOCAL),
                              coord_view(XF))                  # A = x
            nc.sync.dma_start(rD[:].rearrange("p (b s) -> p b s", b=B_LOCAL),
                              coord_view(YF))                  # D = y
            nc.sync.dma_start(rII[:].rearrange("p (b s) -> p b s", b=B_LOCAL),
                              coord_view(X0I))
            V.tensor_copy(rB[:], rII[:])                       # B = x0 (float)
            nc.sync.dma_start(rII[:].rearrange("p (b s) -> p b s", b=B_LOCAL),
                              coord_view(Y0I))
            V.tensor_copy(rC[:], rII[:])                       # C = y0 (float)

            # ---- clips ----
            V.tensor_scalar(rE[:], rB[:], 0.0, float(W - 1), OP.max, OP.min)  # E=x0c
            V.tensor_scalar(rF[:], rB[:], 1.0, float(W - 1), OP.add, OP.min)
            V.tensor_scalar(rF[:], rF[:], 0.0, None, OP.max)                  # F=x1c
            V.tensor_scalar(rG[:], rE[:], float(W - 2), None, OP.min)         # G=xg
            V.tensor_scalar(rB[:], rC[:], 0.0, float(H - 1), OP.max, OP.min)  # B=y0c
            V.tensor_scalar(rHh[:], rC[:], 1.0, float(H - 1), OP.add, OP.min)
            V.tensor_scalar(rHh[:], rHh[:], 0.0, None, OP.max)                # Hh=y1c

            # ---- lerp deltas & weights ----
            V.tensor_tensor(rC[:], rF[:], rA[:], OP.subtract)   # C = hx0 = x1c-x
            V.tensor_tensor(rA[:], rA[:], rE[:], OP.subtract)   # A = hx1 = x-x0c
            V.tensor_tensor(rI[:], rHh[:], rD[:], OP.subtract)  # I = vy0 = y1c-y
            V.tensor_tensor(rD[:], rD[:], rB[:], OP.subtract)   # D = vy1 = y-y0c
            # rB = y0c, rHh = y1c no longer needed (indices precomputed on host)

            V.tensor_tensor(rB[:], rC[:], rI[:], OP.mult)       # B = wa
            V.tensor_tensor(rHh[:], rC[:], rD[:], OP.mult)      # Hh = wb
            V.tensor_tensor(rC[:], rA[:], rI[:], OP.mult)       # C = wc
            V.tensor_tensor(rA[:], rA[:], rD[:], OP.mult)       # A = wd

            # ---- slot positions within the gathered 4-pixel block ----
            # block pixel start = xg - (xg mod 2); sA = x0c-xg+m2, sC = x1c-xg+m2
            # m2 = xg - 2*floor(xg/2); floor(xg/2) = rne(0.5*xg - 0.25) via 2^23
            MAGIC = 8388608.0
            V.tensor_scalar(rI[:], rG[:], 0.5, -0.25, OP.mult, OP.add)
            V.tensor_scalar(rI[:], rI[:], MAGIC, None, OP.add)
            V.tensor_scalar(rI[:], rI[:], MAGIC, None, OP.subtract)  # floor(xg/2)
            V.scalar_tensor_tensor(rI[:], rI[:], -2.0, rG[:],
                                   OP.mult, OP.add)             # I = m2
            V.tensor_tensor(rD[:], rE[:], rG[:], OP.subtract)
            V.tensor_tensor(rD[:], rD[:], rI[:], OP.add)        # D = sA in {0,1,2}
            V.tensor_tensor(rE[:], rF[:], rG[:], OP.subtract)
            V.tensor_tensor(rE[:], rE[:], rI[:], OP.add)        # E = sC in {0,1,2}

            # ---- slot-routed weights: w<pair><k> = w_lo*ind(sA==k)+w_hi*ind(sC==k)
            wsel = wsel_store
            wsel["ident"] = ident
            for k in range(3):
                V.tensor_scalar(rF[:], rD[:], float(k), None, OP.is_equal)  # iA_k
                V.tensor_scalar(rG[:], rE[:], float(k), None, OP.is_equal)  # iC_k
                for pair, (w_lo, w_hi) in (("AC", (rB, rC)), ("BD", (rHh, rA))):
                    wt = pp.tile([P, NJ], f32, name=f"w{pair}{k}", tag=f"w{pair}{k}")
                    V.tensor_tensor(wt[:], w_lo[:], rF[:], OP.mult)
                    V.tensor_tensor(rK[:], w_hi[:], rG[:], OP.mult)
                    V.tensor_tensor(wt[:], wt[:], rK[:], OP.add)
                    wsel[pair, k] = wt

            # ---- gather + weighted-sum chunks ----
            import contextlib
            loop_cm = tc.For_i(0, repeat, 1) if repeat > 1 else \
                contextlib.nullcontext()
            with loop_cm:
                chunk_body(nc, tc, cp, mp, qp, iw, wsel, X, out_v, nq)

    nc.compile()
    return nc


def chunk_body(nc, tc, cp, mp, qp, iw, wsel, X, out_v, nq):
            V = nc.vector
            S = nc.scalar
            ident = wsel["ident"]
            for ch in range(NCH):
                b = ch // (NCH // B_LOCAL)
                co = ch * CH                 # global slot base
                sl = slice(co, co + CH)
                f0 = (co - b * SJ) * 8       # wrapped free offset within batch
                g = {}
                for pi, pair in enumerate(("AC", "BD")):
                    gt = cp.tile([P, CH, 4 * C], f32, name="g" + pair,
                                 tag="g" + pair)
                    in_ap = bass.AP(X.ap().tensor, b * BATCH_ELEMS,
                                    [[2 * C, BATCH_ELEMS // (2 * C)], [1, 4 * C]])
                    nc.gpsimd.dma_gather(
                        out_ap=gt[:],
                        in_ap=in_ap,
                        idxs_ap=iw[pair][:, b * SJ * 8 + f0:
                                         b * SJ * 8 + f0 + CH * 8],
                        num_idxs=NIDX,
                        num_idxs_reg=NIDX,
                        elem_size=4 * C,
                        elem_step=2 * C,
                        single_packet=False,
                        queue_num=(2 * ch + pi) % nq,
                    )
                    g[pair] = gt

                ps = qp.tile([P, CH * C], f32, name="ps", tag="ps")
                first = True
                for pair in ("AC", "BD"):
                    for k in range(3):
                        m = mp.tile([P, CH, C], f32, name=f"m{pair}{k}",
                                    tag=f"m{pair}{k}")
                        V.tensor_tensor(m[:], g[pair][:, :, C * k:C * (k + 1)],
                                        _bcast(wsel[pair, k][:, sl], C), OP.mult)
                        nc.tensor.matmul(out=ps[:], lhsT=ident[:],
                                         rhs=m[:].rearrange("p a b -> p (a b)"),
                                         start=first, stop=(pair == "BD" and k == 2))
                        first = False

                osb = cp.tile([P, CH * C], f32, name="osb", tag="osb")
                S.activation(osb[:], ps[:], ACT.Copy)
                nc.sync.dma_start(out_v[b, :, co - b * SJ:co - b * SJ + CH, :],
                                  osb[:])


_NC_CACHE = []


def _get_nc():
    if not _NC_CACHE:
        _NC_CACHE.append(build_nc())
    return _NC_CACHE[0]


def host_coords(t):
    """Replicate the oracle's coordinate pipeline with the same jax ops on
    the same backend, so x/y/x0/y0 are bitwise identical to the oracle's."""
    import jax.numpy as jnp

    B = t.shape[0]
    xs = jnp.linspace(-1.0, 1.0, WO, dtype=jnp.float32)
    ys = jnp.linspace(-1.0, 1.0, HO, dtype=jnp.float32)
    xc, yc = jnp.meshgrid(xs, ys)
    grid = jnp.stack([xc.ravel(), yc.ravel(), jnp.ones(HWO, jnp.float32)], axis=0)
    theta = jnp.asarray(t).reshape(B, 2, 3)
    sampled = jnp.einsum('bij,jk->bik', theta, grid)
    x = sampled[:, 0, :].reshape(-1)
    y = sampled[:, 1, :].reshape(-1)
    x = 0.5 * (x + 1.0) * jnp.float32(H)
    y = 0.5 * (y + 1.0) * jnp.float32(W)
    x0 = x.astype(jnp.int32)
    y0 = y.astype(jnp.int32)
    return (np.asarray(x), np.asarray(y), np.asarray(x0), np.asarray(y0))


def host_aux(t):
    """Host-side: coords (bitwise oracle replica) + wrapped int16 gather
    indices. Returns dict of per-core input arrays (leading dim N_CORES)."""
    B = t.shape[0]
    xf, yf, x0i, y0i = host_coords(t)

    x0c = np.clip(x0i, 0, W - 1)
    xg = np.minimum(x0c, W - 2)
    y0c = np.clip(y0i, 0, H - 1)
    y1c = np.clip(y0i + 1, 0, H - 1)
    kAC = (y0c * (W // 2) + (xg >> 1)).astype(np.int64)   # block idx, 0..32767
    kBD = (y1c * (W // 2) + (xg >> 1)).astype(np.int64)

    def wrap(k):
        # k: [B*HWO] in pixel-linear order; per batch build wrapped int16:
        # ordinal n = j*128 + p maps to pixel p*SJ + j;
        # wrapped[q, f] = k_by_n[f*16 + q%16], replicated over 8 cores.
        k2 = k.reshape(B, P, SJ)
        k_by_n = np.transpose(k2, (0, 2, 1)).reshape(B, HWO)  # [B, n]
        w = k_by_n.reshape(B, HWO // 16, 16).transpose(0, 2, 1)  # [B, 16, n//16]
        w = np.tile(w, (1, 8, 1)).astype(np.int16)               # [B, 128, n//16]
        return w.reshape(N_CORES, B_LOCAL, P, SJ * 8)

    return {
        "xf": np.ascontiguousarray(xf.reshape(N_CORES, B_LOCAL * HWO)),
        "yf": np.ascontiguousarray(yf.reshape(N_CORES, B_LOCAL * HWO)),
        "x0i": np.ascontiguousarray(x0i.reshape(N_CORES, B_LOCAL * HWO)),
        "y0i": np.ascontiguousarray(y0i.reshape(N_CORES, B_LOCAL * HWO)),
        "iw": np.ascontiguousarray(
            np.stack([wrap(kAC), wrap(kBD)], axis=1)),  # [cores, 2, BL, P, SJ*8]
    }


def shard_X(X):
    Xs = X.reshape(N_CORES, B_LOCAL * BATCH_ELEMS)
    return np.concatenate(
        [Xs, np.zeros((N_CORES, PAD), np.float32)], axis=1)


def kernel(X, t):
    from concourse import bass_utils

    nc = _get_nc()
    B = X.shape[0]
    assert B == N_CORES * B_LOCAL
    aux = host_aux(t)
    Xp = shard_X(np.ascontiguousarray(X))
    in_maps = [dict(X=Xp[i], **{k: v[i] for k, v in aux.items()})
               for i in range(N_CORES)]
    res = bass_utils.run_bass_kernel_spmd(nc, in_maps, core_ids=list(range(N_CORES)))
    out = np.stack([res.results[i]["out"] for i in range(N_CORES)])
    return out.reshape(B, HO, WO, C)
